# revision 1
# baseline (speedup 1.0000x reference)
"""Trainium2 Bass kernel for a GPT-2 style transformer block
(S=3072, E=1024, 16 heads, MLP 4x), distributed over 8 NeuronCores.

Sharding:
  - LN1 runs sequence-parallel (each core normalizes+transposes its 384-row
    chunk), then an AllGather gives every core the full [E, S] normalized,
    transposed activations (bf16).
  - Attention is tensor-parallel over heads (2 heads/core).
  - One AllToAll reshards attention output to sequence-parallel chunks;
    proj + residual + LN2 + the full MLP run per-chunk with no further
    collectives. The host concatenates the 8 output chunks.

On-device layout is "transposed activations" [feature-partition, seq-free]:
every matmul contracts over the partition dim, and the causal softmax needs
no row-max pass (scores are O(1); masked lanes of the diagonal band are
zeroed post-exp by a gpsimd affine_select; fully-masked blocks are skipped).
The softmax denominator comes free from a ones-augmented V column. Weights
are pre-tiled on the host so every DMA is contiguous. Matmuls run as
float32r (scores/AV/proj) and bfloat16 (qkv, MLP) with fp32 PSUM
accumulation.
"""

import numpy as np

E, H, I = 1024, 16, 4096
W = 8
MASK = -10000.0

_CACHE = {}


def _build(SS: int, dt_mm_name: str, mock_cc: bool = False):
    """Build the SPMD Bass program for sequence length SS.
    dt_mm_name: 'float32r' (fast) or 'float32' (exact) for the fp32-operand
    matmuls (scores, AV, proj)."""
    import concourse.mybir as mybir
    import concourse.tile as tile
    from concourse import bacc
    from concourse.masks import make_identity

    f32 = mybir.dt.float32
    bf16 = mybir.dt.bfloat16
    dt_mm = getattr(mybir.dt, dt_mm_name)
    AF = mybir.ActivationFunctionType
    ALU = mybir.AluOpType
    X = mybir.AxisListType.X

    CH = SS // W          # seq rows per core
    NB = SS // 128        # 128-blocks along full sequence
    B = CH // 128         # 128-blocks per chunk

    nc = bacc.Bacc(None)

    hid = nc.dram_tensor("hidden", [CH, E], f32, kind="ExternalInput")
    qkv_w = nc.dram_tensor("qkv_w", [128, 3 * 8 * 128], bf16, kind="ExternalInput")
    qkv_b = nc.dram_tensor("qkv_b", [128, 3], f32, kind="ExternalInput")
    proj_w = nc.dram_tensor("proj_w", [8 * 128, 8 * 128], dt_mm, kind="ExternalInput")
    proj_b = nc.dram_tensor("proj_b", [128, 8], f32, kind="ExternalInput")
    ln1_w = nc.dram_tensor("ln1_w", [128, 8], f32, kind="ExternalInput")
    ln1_b = nc.dram_tensor("ln1_b", [128, 8], f32, kind="ExternalInput")
    ln2_w = nc.dram_tensor("ln2_w", [128, 8], f32, kind="ExternalInput")
    ln2_b = nc.dram_tensor("ln2_b", [128, 8], f32, kind="ExternalInput")
    w1 = nc.dram_tensor("w1", [32 * 128, 8 * 128], bf16, kind="ExternalInput")
    b1 = nc.dram_tensor("b1", [128, 32], f32, kind="ExternalInput")
    w2 = nc.dram_tensor("w2", [8 * 128, 32 * 128], bf16, kind="ExternalInput")
    b2 = nc.dram_tensor("b2", [128, 8], f32, kind="ExternalInput")
    mask_b = nc.dram_tensor("mask_bias", [128, NB], f32, kind="ExternalInput")
    out = nc.dram_tensor("out", [CH, E], f32, kind="ExternalOutput")

    rg = [list(range(W))]

    with tile.TileContext(nc) as tc:
        with (
            tc.tile_pool(name="dram", bufs=1, space="DRAM") as dram,
            tc.tile_pool(name="const", bufs=1) as const,
            tc.tile_pool(name="persist", bufs=1) as persist,
            tc.tile_pool(name="work", bufs=2) as work,
            tc.tile_pool(name="xgp", bufs=8) as xgp,
            tc.tile_pool(name="exp", bufs=4) as exp_pool,
            tc.tile_pool(name="psum", bufs=2, space="PSUM") as psum,
            tc.tile_pool(name="psacc", bufs=2, space="PSUM") as psacc,
        ):
            # ----- constants -----
            ident = const.tile([128, 128], f32, tag="ident", name="ident")
            make_identity(nc, ident[:])
            ident_bf = const.tile([128, 128], bf16, tag="identbf", name="identbf")
            nc.vector.tensor_copy(ident_bf[:], ident[:])
            eps_sb = const.tile([128, 1], f32, tag="eps", name="eps")
            nc.vector.memset(eps_sb[:], 1e-5)

            def load2d(dram_t, shape, name):
                t = const.tile(shape, f32, tag=name, name=name)
                nc.sync.dma_start(out=t[:], in_=dram_t[:, :])
                return t

            ln1_w_sb = load2d(ln1_w, [128, 8], "ln1w")
            ln1_b_sb = load2d(ln1_b, [128, 8], "ln1b")
            ln2_w_sb = load2d(ln2_w, [128, 8], "ln2w")
            ln2_b_sb = load2d(ln2_b, [128, 8], "ln2b")
            qkv_b_sb = load2d(qkv_b, [128, 3], "qkvb")
            proj_b_sb = load2d(proj_b, [128, 8], "projb")
            b1_sb = load2d(b1, [128, 32], "b1")
            b2_sb = load2d(b2, [128, 8], "b2")
            mb_sb = load2d(mask_b, [128, NB], "maskbias")

            def dma(out_, in_):
                return nc.sync.dma_start(out=out_, in_=in_)

            # ----- LN (row layout) + transpose; w/b applied post-transpose -----
            def layer_norm_T(x_tiles, w_sb, b_sb, out_dt):
                xT = [persist.tile([128, CH], out_dt, tag=f"lnT{k}", name=f"lnT{k}")
                      for k in range(8)]
                for t in range(len(x_tiles)):
                    x = x_tiles[t]
                    stat = work.tile([128, 8], f32, tag="lnstat", name="lnstat")
                    scr = work.tile([128, E], f32, tag="lnscr", name="lnscr")
                    nc.vector.reduce_sum(out=stat[:, 0:1], in_=x[:], axis=X)
                    nc.vector.tensor_scalar_mul(stat[:, 1:2], stat[:, 0:1], 1.0 / E)
                    nc.scalar.activation(scr[:], x[:], AF.Square,
                                         accum_out=stat[:, 2:3])
                    nc.vector.tensor_scalar_mul(stat[:, 2:3], stat[:, 2:3], 1.0 / E)
                    nc.vector.tensor_tensor(out=stat[:, 3:4], in0=stat[:, 1:2],
                                            in1=stat[:, 1:2], op=ALU.mult)
                    nc.vector.tensor_tensor(out=stat[:, 3:4], in0=stat[:, 2:3],
                                            in1=stat[:, 3:4], op=ALU.subtract)
                    nc.scalar.activation(stat[:, 4:5], stat[:, 3:4], AF.Sqrt,
                                         bias=eps_sb[:], scale=1.0)
                    nc.vector.reciprocal(out=stat[:, 4:5], in_=stat[:, 4:5])
                    nc.vector.tensor_tensor(out=stat[:, 5:6], in0=stat[:, 1:2],
                                            in1=stat[:, 4:5], op=ALU.mult)
                    nc.vector.tensor_scalar_mul(stat[:, 5:6], stat[:, 5:6], -1.0)
                    xn = work.tile([128, E], f32, tag="lnscr", name="lnxn")
                    nc.vector.tensor_scalar(out=xn[:], in0=x[:],
                                            scalar1=stat[:, 4:5],
                                            scalar2=stat[:, 5:6],
                                            op0=ALU.mult, op1=ALU.add)
                    for m in range(8):
                        tp = psum.tile([128, 128], f32, tag="tp", name="tp")
                        nc.tensor.transpose(tp[:], xn[:, m * 128:(m + 1) * 128],
                                            ident[:])
                        nc.vector.tensor_scalar(
                            out=xT[m][:, t * 128:(t + 1) * 128], in0=tp[:],
                            scalar1=w_sb[:, m:m + 1], scalar2=b_sb[:, m:m + 1],
                            op0=ALU.mult, op1=ALU.add)
                return xT

            # ----- stage 1: LN1 on own chunk -----
            x_rows = []
            for t in range(B):
                xt = persist.tile([128, E], f32, tag=f"xrow{t}", name=f"xrow{t}")
                dma(xt[:], hid[t * 128:(t + 1) * 128, :])
                x_rows.append(xt)
            xnT = layer_norm_T(x_rows, ln1_w_sb, ln1_b_sb, bf16)

            # ----- stage 2: AllGather normalized-transposed chunks (bf16) -----
            ag_in = dram.tile([E, CH], bf16)
            ag_out = dram.tile([W * E, CH], bf16, addr_space="Shared")
            for m in range(8):
                dma(ag_in[m * 128:(m + 1) * 128, :], xnT[m][:])
            if mock_cc:
                nc.sync.dma_start(out=ag_out[0:E, :], in_=ag_in[:, :])
            else:
                nc.gpsimd.collective_compute(
                    "AllGather", ALU.bypass, replica_groups=rg,
                    ins=[ag_in.opt()], outs=[ag_out.opt()])

            # ----- stage 3: qkv for this core's 2 heads (bf16 matmuls) -----
            wqkv = persist.tile([128, 3 * 8 * 128], bf16, tag="wqkv", name="wqkv")
            dma(wqkv[:], qkv_w[:, :])

            qT = persist.tile([128, SS], dt_mm, tag="qT", name="qT")
            kT = persist.tile([128, SS], dt_mm, tag="kT", name="kT")
            vT = persist.tile([128, SS], f32, tag="vT", name="vT")
            qkvT = [qT, kT, vT]
            for j in range(W):
                xg = [xgp.tile([128, CH], bf16, tag="xg", name="xg")
                      for _ in range(8)]
                for k in range(8):
                    dma(xg[k][:], ag_out[j * E + k * 128:j * E + (k + 1) * 128, :])
                for c in range(3):
                    ps = psacc.tile([128, CH], f32, tag="mmacc", name="mmacc")
                    for k in range(8):
                        nc.tensor.matmul(
                            ps[:],
                            lhsT=wqkv[:, (c * 8 + k) * 128:(c * 8 + k + 1) * 128],
                            rhs=xg[k][:],
                            start=(k == 0), stop=(k == 7))
                    nc.scalar.activation(
                        qkvT[c][:, j * CH:(j + 1) * CH], ps[:], AF.Identity,
                        bias=qkv_b_sb[:, c:c + 1], scale=1.0)

            # ----- stage 4: V transposed + ones-augmented column -----
            v_aug = [persist.tile([128, NB * 65], dt_mm, tag=f"vaug{h}",
                                  name=f"vaug{h}") for h in range(2)]
            ones128 = nc.const_aps.tensor(1.0, (128, 1))
            for h in range(2):
                for tb in range(NB):
                    nc.scalar.copy(
                        v_aug[h][:, tb * 65 + 64:tb * 65 + 65], ones128)
            for tb in range(NB):
                # one transpose covers both heads: out rows=t, cols=(h0 e, h1 e)
                tp = psum.tile([128, 128], f32, tag="tp", name="vtp")
                nc.tensor.transpose(tp[:], vT[:, tb * 128:(tb + 1) * 128], ident[:])
                for h in range(2):
                    nc.vector.tensor_copy(v_aug[h][:, tb * 65:tb * 65 + 64],
                                          tp[:, 64 * h:64 * h + 64])

            # ----- stage 5: attention (h-outer) + per-head AllToAll so the
            # head-0 collective overlaps head-1 compute -----
            a2a_in = [dram.tile([W * 64, CH], dt_mm, name=f"a2a_in{h}")
                      for h in range(2)]
            a2a_out = [dram.tile([W * 64, CH], dt_mm, name=f"a2a_out{h}")
                       for h in range(2)]
            for h in range(2):
                e0 = 64 * h
                for j in range(W):
                    n_t = B * (j + 1)
                    av = psacc.tile([65, CH], f32, tag="avacc", name="avacc")
                    for tb in range(n_t):
                        sc = psum.tile([128, CH], f32, tag="sc", name="sc")
                        nc.tensor.matmul(
                            sc[:],
                            lhsT=kT[e0:e0 + 64, tb * 128:(tb + 1) * 128],
                            rhs=qT[e0:e0 + 64, j * CH:(j + 1) * CH],
                            start=True, stop=True)
                        ex = exp_pool.tile([128, CH], dt_mm, tag="ex", name="ex")
                        nc.scalar.activation(ex[:], sc[:], AF.Exp,
                                             bias=mb_sb[:, tb:tb + 1], scale=1.0)
                        p = tb - B * j
                        if p >= 0:
                            # zero the future (t > s) lanes of the diagonal band
                            nc.gpsimd.affine_select(
                                out=ex[:], in_=ex[:],
                                compare_op=ALU.is_ge,   # keep where s-t-128p >= 0
                                fill=0.0,
                                base=-128 * p,
                                channel_multiplier=-1,
                                pattern=[[1, CH]],
                            )
                        nc.tensor.matmul(
                            av[:],
                            lhsT=v_aug[h][:, tb * 65:(tb + 1) * 65],
                            rhs=ex[:],
                            start=(tb == 0), stop=(tb == n_t - 1))
                    recip = work.tile([1, CH], f32, tag="recip", name="recip")
                    nc.vector.reciprocal(out=recip[:], in_=av[64:65, :])
                    bc = work.tile([64, CH], f32, tag="bc", name="bc")
                    nc.gpsimd.partition_broadcast(bc[:], recip[:])
                    avs = work.tile([64, CH], dt_mm, tag="avsc", name="avsc")
                    nc.vector.tensor_tensor(out=avs[:], in0=av[0:64, :],
                                            in1=bc[:], op=ALU.mult)
                    dma(a2a_in[h][j * 64:(j + 1) * 64, :], avs[:])
                # ----- stage 6: AllToAll for this head's slices -----
                if mock_cc:
                    nc.sync.dma_start(out=a2a_out[h][:, :], in_=a2a_in[h][:, :])
                else:
                    nc.gpsimd.collective_compute(
                        "AllToAll", ALU.bypass, replica_groups=rg,
                        ins=[a2a_in[h].opt()], outs=[a2a_out[h].opt()])

            # ----- stage 7: proj + residual -----
            aT = [persist.tile([128, CH], dt_mm, tag=f"aT{k}", name=f"aT{k}")
                  for k in range(8)]
            for k in range(8):
                dma(aT[k][0:64, :], a2a_out[0][k * 64:(k + 1) * 64, :])
                dma(aT[k][64:128, :], a2a_out[1][k * 64:(k + 1) * 64, :])
            res1 = [persist.tile([128, E], f32, tag=f"res1_{t}", name=f"res1_{t}")
                    for t in range(B)]
            for m in range(8):
                wpm = work.tile([128, 8 * 128], dt_mm, tag="wpm", name="wpm",
                                bufs=3)
                dma(wpm[:], proj_w[m * 128:(m + 1) * 128, :])
                ps = psacc.tile([128, CH], f32, tag="mmacc", name="mmacc")
                for k in range(8):
                    nc.tensor.matmul(
                        ps[:], lhsT=wpm[:, k * 128:(k + 1) * 128],
                        rhs=aT[k][:], start=(k == 0), stop=(k == 7))
                pTm = work.tile([128, CH], f32, tag="pTm", name="pTm")
                nc.vector.tensor_scalar(
                    out=pTm[:], in0=ps[:], scalar1=proj_b_sb[:, m:m + 1],
                    scalar2=None, op0=ALU.add)
                for t in range(B):
                    tp = psum.tile([128, 128], f32, tag="tp", name="tp")
                    nc.tensor.transpose(tp[:], pTm[:, t * 128:(t + 1) * 128],
                                        ident[:])
                    nc.vector.tensor_tensor(
                        out=res1[t][:, m * 128:(m + 1) * 128],
                        in0=tp[:], in1=x_rows[t][:, m * 128:(m + 1) * 128],
                        op=ALU.add)

            # ----- stage 8: LN2 (output tiles bf16 for the MLP) -----
            l2T = layer_norm_T(res1, ln2_w_sb, ln2_b_sb, bf16)

            # ----- stage 9: MLP (full, on this core's seq chunk; bf16) -----
            h1T = [persist.tile([128, CH], bf16, tag=f"h1T{m}", name=f"h1T{m}")
                   for m in range(32)]
            for m in range(32):
                w1m = work.tile([128, 8 * 128], bf16, tag="w1m", name="w1m",
                                bufs=4)
                dma(w1m[:], w1[m * 128:(m + 1) * 128, :])
                ps = psacc.tile([128, CH], f32, tag="mmacc", name="mmacc")
                for k in range(8):
                    nc.tensor.matmul(
                        ps[:], lhsT=w1m[:, k * 128:(k + 1) * 128],
                        rhs=l2T[k][:], start=(k == 0), stop=(k == 7))
                nc.vector.tensor_scalar(
                    out=h1T[m][:], in0=ps[:], scalar1=b1_sb[:, m:m + 1],
                    scalar2=0.0, op0=ALU.add, op1=ALU.max)

            oT = [persist.tile([128, CH], bf16, tag=f"oT{m}", name=f"oT{m}")
                  for m in range(8)]
            for m in range(8):
                ps = psacc.tile([128, CH], f32, tag="mmacc", name="mmacc")
                for half in range(2):
                    w2m = work.tile([128, 16 * 128], bf16, tag="w2m", name="w2m", bufs=3)
                    dma(w2m[:], w2[m * 128:(m + 1) * 128,
                                   half * 16 * 128:(half + 1) * 16 * 128])
                    for kk in range(16):
                        k = half * 16 + kk
                        nc.tensor.matmul(
                            ps[:], lhsT=w2m[:, kk * 128:(kk + 1) * 128],
                            rhs=h1T[k][:], start=(k == 0), stop=(k == 31))
                nc.vector.tensor_scalar(
                    out=oT[m][:], in0=ps[:], scalar1=b2_sb[:, m:m + 1],
                    scalar2=None, op0=ALU.add)

            # ----- stage 10: transpose back + final residual + out -----
            for t in range(B):
                orow = work.tile([128, E], f32, tag="orow", name="orow")
                for m in range(8):
                    tp = psum.tile([128, 128], bf16, tag="tp", name="tp")
                    nc.tensor.transpose(tp[:], oT[m][:, t * 128:(t + 1) * 128],
                                        ident_bf[:])
                    nc.vector.tensor_tensor(
                        out=orow[:, m * 128:(m + 1) * 128],
                        in0=tp[:], in1=res1[t][:, m * 128:(m + 1) * 128],
                        op=ALU.add)
                dma(out[t * 128:(t + 1) * 128, :], orow[:])

    return nc


def _prepare_in_maps(inputs, SS: int):
    """Host-side prep: slice per core, prescale q by 1/8, pre-tile all weight
    matrices so every device DMA is contiguous; bf16-cast qkv/mlp weights."""
    import ml_dtypes

    bf16 = ml_dtypes.bfloat16
    CH = SS // W
    NB = SS // 128
    hid = np.ascontiguousarray(
        np.asarray(inputs["hidden_states"], np.float32)[0, :SS])
    attn_w = np.asarray(inputs["attn_w"], np.float32).copy()
    attn_b = np.asarray(inputs["attn_b"], np.float32).copy()
    attn_w[:, :E] *= 0.125
    attn_b[:E] *= 0.125
    mask = np.asarray(inputs["mask"])[0, 0, 0, :SS]
    mask_bias = np.where(mask, 0.0, MASK).astype(np.float32)

    def vec2d(v, n):
        return np.ascontiguousarray(
            np.asarray(v, np.float32)[:n].reshape(n // 128, 128).T)

    proj_w = np.asarray(inputs["proj_w"], np.float32)
    w1 = np.asarray(inputs["mlp_w1"], np.float32)
    w2 = np.asarray(inputs["mlp_w2"], np.float32)

    # X[k*128+p, m*128+f] -> [(m p), (k f)]
    def tile_mk(x, km, mm_):
        return np.ascontiguousarray(
            x.reshape(km, 128, mm_, 128).transpose(2, 1, 0, 3)
            .reshape(mm_ * 128, km * 128))

    common = {
        "proj_w": tile_mk(proj_w, 8, 8),
        "proj_b": vec2d(inputs["proj_b"], E),
        "ln1_w": vec2d(inputs["ln1_w"], E),
        "ln1_b": vec2d(inputs["ln1_b"], E),
        "ln2_w": vec2d(inputs["ln2_w"], E),
        "ln2_b": vec2d(inputs["ln2_b"], E),
        "w1": tile_mk(w1, 8, 32).astype(bf16),
        "b1": vec2d(inputs["mlp_b1"], I),
        "w2": tile_mk(w2, 32, 8).astype(bf16),
        "b2": vec2d(inputs["mlp_b2"], E),
        "mask_bias": np.ascontiguousarray(mask_bias.reshape(NB, 128).T),
    }
    in_maps = []
    for i in range(W):
        wq = np.empty((128, 3, 8, 128), np.float32)
        bq = np.empty((128, 3), np.float32)
        for c in range(3):
            cols = slice(c * E + 128 * i, c * E + 128 * i + 128)
            wq[:, c] = attn_w[:, cols].reshape(8, 128, 128).transpose(1, 0, 2)
            bq[:, c] = attn_b[cols]
        in_maps.append({
            "hidden": np.ascontiguousarray(hid[i * CH:(i + 1) * CH]),
            "qkv_w": np.ascontiguousarray(wq.reshape(128, -1)).astype(bf16),
            "qkv_b": np.ascontiguousarray(bq),
            **common,
        })
    return in_maps


def _run(inputs, SS, dt_mm, **kw):
    from concourse.bass_utils import run_bass_kernel_spmd

    key = (SS, dt_mm)
    if key not in _CACHE:
        nc = _build(SS, dt_mm)
        nc.finalize()
        _CACHE[key] = nc
    nc = _CACHE[key]
    in_maps = _prepare_in_maps(inputs, SS)
    res = run_bass_kernel_spmd(nc, in_maps, core_ids=list(range(W)), **kw)
    full = np.concatenate([r["out"] for r in res.results], axis=0)
    return full[None].astype(np.float32), res


def kernel(**inputs) -> np.ndarray:
    out, _ = _run(inputs, 3072, "float32r")
    return out

